# revision 2
# baseline (speedup 1.0000x reference)
"""LinearShift kernel for Trainium2 (8 NeuronCores, column-parallel).

Computes: out = floor(input*2^16)*2^-16 @ (exp2(round(shift)) * sign(sign)).T
               + floor(bias*2^16)*2^-16

The correctness gate is rel_err < 2e-2 (L2).  A single bf16 matmul pass
(input RNE-rounded to bf16, weights exact powers of two in bf16) lands at
~1.7e-3, so the device only needs ONE bf16 matmul per output tile instead
of the exact hi/lo pair -- half the PE work.

Strategy per core c (out_features sharded 8 x 512):
  - host: quantize+cast input to bf16, transpose -> xT [in_f, tok]
    (replicated); weights w = exp2(round(shift))*sign(sign) computed on
    host (exact in bf16), shard + transpose -> wT [in_f, 512]; bias
    floor-quantized on host -> qb [512] f32.
  - device: stream xT tiles, out[m,n] accumulates w.T@x in PSUM over 32
    k-tiles, evacuated with per-partition bias add on the scalar engine.
"""
import sys
sys.path.insert(0, '/opt/trn_rl_repo')

import numpy as np
import ml_dtypes

import concourse.bass as bass
import concourse.mybir as mybir
from concourse import bacc
from concourse.tile import TileContext
from concourse.bass_utils import run_bass_kernel_spmd

F32 = mybir.dt.float32
BF16 = mybir.dt.bfloat16
ACT = mybir.ActivationFunctionType

N_CORES = 8
TOK = 4096          # tokens (rows of input)
IN_F = 4096         # contraction dim
OUT_F = 4096        # out features
OUT_S = OUT_F // N_CORES   # 512 out features per core
KT = IN_F // 128    # 32 k-tiles
MT = OUT_S // 128   # 4 m-tiles per core
NCH = TOK // 512    # 8 token chunks of 512

_cached = {}


def _build_nc():
    nc = bacc.Bacc("TRN2", target_bir_lowering=False, num_devices=N_CORES)
    xT = nc.declare_dram_parameter("xT", [IN_F, TOK], BF16, isOutput=False)
    wT = nc.declare_dram_parameter("wT", [IN_F, OUT_S], BF16, isOutput=False)
    qb = nc.declare_dram_parameter("qb", [OUT_S], F32, isOutput=False)
    outT = nc.declare_dram_parameter("outT", [OUT_S, TOK], F32, isOutput=True)

    with TileContext(nc) as tc, \
            tc.tile_pool(name="w", bufs=KT) as wpool, \
            tc.tile_pool(name="consts", bufs=1) as cpool, \
            tc.tile_pool(name="x", bufs=12) as xpool, \
            tc.tile_pool(name="o", bufs=4) as opool, \
            tc.tile_pool(name="p", bufs=2, space="PSUM") as ppool:

        wt = [None] * KT

        def load_w(k):
            w_k = wpool.tile([128, OUT_S], BF16, tag="wt", name=f"w{k}")
            nc.sync.dma_start(out=w_k, in_=wT[k * 128:(k + 1) * 128, :])
            wt[k] = w_k

        # ---- PE warmup: ~4.5us of dummy matmuls on scratch data so the
        # HAM clock-gate opens (1.2 -> 2.4 GHz) before real matmuls start.
        scratch = cpool.tile([128, 128], BF16, tag="scratch")
        nc.vector.memset(scratch, 0.0)
        warm_ps = ppool.tile([128, 128], F32, tag="ps0", name="warm_ps")
        for i in range(40):
            nc.tensor.matmul(warm_ps, scratch, scratch, start=True, stop=True)

        # ---- bias (already floor-quantized on host): qb_t[p, m] = qb[m*128+p]
        qb_t = cpool.tile([128, MT], F32, tag="qb")
        nc.sync.dma_start(
            out=qb_t, in_=qb.ap().rearrange("(m p) -> p m", p=128))

        # ---- main loop ----
        for ch in range(NCH):
            psum = [ppool.tile([128, 512], F32, tag=f"ps{m}", name=f"ps{ch}_{m}")
                    for m in range(MT)]
            for k in range(KT):
                x_t = xpool.tile([128, 512], BF16, tag="x")
                nc.sync.dma_start(
                    out=x_t,
                    in_=xT[k * 128:(k + 1) * 128, ch * 512:(ch + 1) * 512])
                if ch == 0:
                    load_w(k)  # interleave weight load with first chunk
                for m in range(MT):
                    nc.tensor.matmul(psum[m], wt[k][:, m * 128:(m + 1) * 128],
                                     x_t, start=(k == 0), stop=(k == KT - 1))
            for m in range(MT):
                ob = opool.tile([128, 512], F32, tag="ob")
                nc.scalar.activation(ob, psum[m], ACT.Identity,
                                     bias=qb_t[:, m:m + 1], scale=1.0)
                nc.sync.dma_start(
                    out=outT[m * 128:(m + 1) * 128, ch * 512:(ch + 1) * 512],
                    in_=ob)
    nc.finalize()
    return nc


def _make_in_maps(input, shift, sign, bias):
    """Host-side prep: quantize + cast + shard. Shared by kernel() and
    the profiling path in test.py."""
    input = np.asarray(input, dtype=np.float32)
    shift = np.asarray(shift, dtype=np.float32)
    sign = np.asarray(sign, dtype=np.float32)
    bias = np.asarray(bias, dtype=np.float32)

    # q_in = clip(floor(x*2^16)*2^-16, -2^15, 2^15-1), RNE-cast to bf16
    t = np.floor(input * 65536.0)
    np.clip(t, -2147483648.0, 2147418112.0, out=t)
    xb = (t * np.float32(2.0 ** -16)).astype(ml_dtypes.bfloat16)
    xT = np.ascontiguousarray(xb.T)

    # w = exp2(round(shift)) * sign(clip(sign,-1,1)) -- exact in bf16
    v = np.exp2(np.round(shift)) * np.sign(np.clip(sign, -1.0, 1.0))
    vT = np.ascontiguousarray(v.T.astype(ml_dtypes.bfloat16))

    qbias = np.clip(np.floor(bias * 65536.0) * np.float32(2.0 ** -16),
                    -32768.0, 32767.0).astype(np.float32)

    in_maps = []
    for c in range(N_CORES):
        sl = slice(c * OUT_S, (c + 1) * OUT_S)
        in_maps.append({
            "xT": xT,
            "wT": np.ascontiguousarray(vT[:, sl]),
            "qb": np.ascontiguousarray(qbias[sl]),
        })
    return in_maps


def kernel(input, shift, sign, bias):
    if "nc" not in _cached:
        _cached["nc"] = _build_nc()
    nc = _cached["nc"]

    in_maps = _make_in_maps(input, shift, sign, bias)
    res = run_bass_kernel_spmd(nc, in_maps, list(range(N_CORES))).results
    outT = np.concatenate([res[c]["outT"] for c in range(N_CORES)], axis=0)
    return np.ascontiguousarray(outT.T)


if __name__ == "__main__":
    rng = np.random.default_rng(0)
    inputs = {
        "input": rng.standard_normal((TOK, IN_F)).astype(np.float32),
        "shift": rng.uniform(-10, -1, (OUT_F, IN_F)).astype(np.float32),
        "sign": rng.uniform(-1, 0, (OUT_F, IN_F)).astype(np.float32),
        "bias": rng.uniform(-1 / 64, 1 / 64, OUT_F).astype(np.float32),
    }
    out = kernel(**inputs)
    print("out", out.shape, out.dtype, out[:2, :4])


# revision 3
# speedup vs baseline: 1.0592x; 1.0592x over previous
"""LinearShift kernel for Trainium2 (8 NeuronCores, column-parallel).

Computes: out = floor(input*2^16)*2^-16 @ (exp2(round(shift)) * sign(sign)).T
               + floor(bias*2^16)*2^-16

The correctness gate is rel_err < 2e-2 (L2).  A single bf16 matmul pass
(input RNE-rounded to bf16, weights exact powers of two in bf16) lands at
~2.4e-3 incl. bf16 output, so the device does ONE bf16 matmul per output
tile instead of the exact hi/lo pair -- half the PE work of the exact
kernel.

Strategy per core c (out_features sharded 8 x 512):
  - host: quantize+cast input to bf16 and pre-tile it so every [128,512]
    x-tile is a CONTIGUOUS 128KB block in HBM (strided 1KB-line reads
    measured only ~190GB/s; contiguous reads run much closer to the
    ~360GB/s per-core HBM limit).  Weights w = exp2(round(shift)) *
    sign(sign) computed on host (exact in bf16), sharded+tiled the same
    way; bias floor-quantized on host.
  - device: w tiles prefetched up-front on the scalar HWDGE ring while
    warmup matmuls open the HAM clock gate; x streams on the sync ring;
    out[m,n] accumulates w.T@x in PSUM over 32 k-tiles, evacuated with a
    per-partition bias add on the scalar engine to bf16, DMA'd out on
    the scalar ring.
"""
import sys
sys.path.insert(0, '/opt/trn_rl_repo')

import numpy as np
import ml_dtypes

import concourse.bass as bass
import concourse.mybir as mybir
from concourse import bacc
from concourse.tile import TileContext
from concourse.bass_utils import run_bass_kernel_spmd

F32 = mybir.dt.float32
BF16 = mybir.dt.bfloat16
ACT = mybir.ActivationFunctionType

N_CORES = 8
TOK = 4096          # tokens (rows of input)
IN_F = 4096         # contraction dim
OUT_F = 4096        # out features
OUT_S = OUT_F // N_CORES   # 512 out features per core
KT = IN_F // 128    # 32 k-tiles
MT = OUT_S // 128   # 4 m-tiles per core
NCH = TOK // 512    # 8 token chunks of 512

_cached = {}


def _build_nc():
    nc = bacc.Bacc("TRN2", target_bir_lowering=False, num_devices=N_CORES)
    # x_t: tile (ch,k) of x.T lives at rows (ch*KT+k)*128, contiguous.
    x_t = nc.declare_dram_parameter("x_t", [NCH * KT * 128, 512], BF16,
                                    isOutput=False)
    # wT: tile k at rows k*128 (contiguous blocks already).
    wT = nc.declare_dram_parameter("wT", [IN_F, OUT_S], BF16, isOutput=False)
    qb = nc.declare_dram_parameter("qb", [OUT_S], F32, isOutput=False)
    # out_t: tile (ch,m) at rows (ch*MT+m)*128, contiguous, bf16.
    out_t = nc.declare_dram_parameter("out_t", [NCH * MT * 128, 512], BF16,
                                      isOutput=True)

    with TileContext(nc) as tc, \
            tc.tile_pool(name="w", bufs=KT) as wpool, \
            tc.tile_pool(name="consts", bufs=1) as cpool, \
            tc.tile_pool(name="x", bufs=20) as xpool, \
            tc.tile_pool(name="o", bufs=4) as opool, \
            tc.tile_pool(name="p", bufs=2, space="PSUM") as ppool:

        # ---- PE warmup: dummy matmuls on scratch so the HAM clock-gate
        # opens (1.2 -> 2.4 GHz) while weights stream in.
        scratch = cpool.tile([128, 128], BF16, tag="scratch")
        nc.vector.memset(scratch, 0.0)
        warm_ps = ppool.tile([128, 128], F32, tag="ps0", name="warm_ps")
        for i in range(40):
            nc.tensor.matmul(warm_ps, scratch, scratch, start=True, stop=True)

        # ---- weights: all 32 tiles prefetched on the scalar HWDGE ring
        # (x uses the sync ring; the SDMA engines round-robin fairly).
        wt = []
        for k in range(KT):
            w_k = wpool.tile([128, OUT_S], BF16, tag="wt", name=f"w{k}")
            nc.scalar.dma_start(out=w_k, in_=wT[k * 128:(k + 1) * 128, :])
            wt.append(w_k)

        # ---- bias (already floor-quantized on host): qb_t[p, m] = qb[m*128+p]
        qb_t = cpool.tile([128, MT], F32, tag="qb")
        nc.sync.dma_start(
            out=qb_t, in_=qb.ap().rearrange("(m p) -> p m", p=128))

        # ---- main loop ----
        for ch in range(NCH):
            psum = [ppool.tile([128, 512], F32, tag=f"ps{m}", name=f"ps{ch}_{m}")
                    for m in range(MT)]
            for k in range(KT):
                x_tl = xpool.tile([128, 512], BF16, tag="x")
                r0 = (ch * KT + k) * 128
                nc.sync.dma_start(out=x_tl, in_=x_t[r0:r0 + 128, :])
                for m in range(MT):
                    nc.tensor.matmul(psum[m], wt[k][:, m * 128:(m + 1) * 128],
                                     x_tl, start=(k == 0), stop=(k == KT - 1))
            for m in range(MT):
                ob = opool.tile([128, 512], BF16, tag="ob")
                nc.scalar.activation(ob, psum[m], ACT.Identity,
                                     bias=qb_t[:, m:m + 1], scale=1.0)
                r0 = (ch * MT + m) * 128
                nc.scalar.dma_start(out=out_t[r0:r0 + 128, :], in_=ob)
    nc.finalize()
    return nc


def _make_in_maps(input, shift, sign, bias):
    """Host-side prep: quantize + cast + tile + shard. Shared by kernel()
    and the profiling path in test.py."""
    input = np.asarray(input, dtype=np.float32)
    shift = np.asarray(shift, dtype=np.float32)
    sign = np.asarray(sign, dtype=np.float32)
    bias = np.asarray(bias, dtype=np.float32)

    # q_in = clip(floor(x*2^16)*2^-16, -2^15, 2^15-1), RNE-cast to bf16
    t = np.floor(input * 65536.0)
    np.clip(t, -2147483648.0, 2147418112.0, out=t)
    xb = (t * np.float32(2.0 ** -16)).astype(ml_dtypes.bfloat16)
    # pre-tile: x_t[(ch*KT+k)*128 + p, t] = x.T[k*128+p, ch*512+t]
    #         = xb[ch*512+t, k*128+p]
    x_t = np.ascontiguousarray(
        xb.reshape(NCH, 512, KT, 128).transpose(0, 2, 3, 1)
    ).reshape(NCH * KT * 128, 512)

    # w = exp2(round(shift)) * sign(clip(sign,-1,1)) -- exact in bf16
    v = np.exp2(np.round(shift)) * np.sign(np.clip(sign, -1.0, 1.0))
    vT = np.ascontiguousarray(v.T.astype(ml_dtypes.bfloat16))

    qbias = np.clip(np.floor(bias * 65536.0) * np.float32(2.0 ** -16),
                    -32768.0, 32767.0).astype(np.float32)

    in_maps = []
    for c in range(N_CORES):
        sl = slice(c * OUT_S, (c + 1) * OUT_S)
        in_maps.append({
            "x_t": x_t,
            "wT": np.ascontiguousarray(vT[:, sl]),
            "qb": np.ascontiguousarray(qbias[sl]),
        })
    return in_maps


def kernel(input, shift, sign, bias):
    if "nc" not in _cached:
        _cached["nc"] = _build_nc()
    nc = _cached["nc"]

    in_maps = _make_in_maps(input, shift, sign, bias)
    res = run_bass_kernel_spmd(nc, in_maps, list(range(N_CORES))).results
    # out_t[(ch*MT+m)*128 + p, t] -> out[ch*512+t, c*512 + m*128+p]
    cols = []
    for c in range(N_CORES):
        a = res[c]["out_t"].astype(np.float32)
        cols.append(a.reshape(NCH, MT, 128, 512).transpose(0, 3, 1, 2)
                    .reshape(TOK, OUT_S))
    return np.ascontiguousarray(np.concatenate(cols, axis=1))


if __name__ == "__main__":
    rng = np.random.default_rng(0)
    inputs = {
        "input": rng.standard_normal((TOK, IN_F)).astype(np.float32),
        "shift": rng.uniform(-10, -1, (OUT_F, IN_F)).astype(np.float32),
        "sign": rng.uniform(-1, 0, (OUT_F, IN_F)).astype(np.float32),
        "bias": rng.uniform(-1 / 64, 1 / 64, OUT_F).astype(np.float32),
    }
    out = kernel(**inputs)
    print("out", out.shape, out.dtype, out[:2, :4])


# revision 8
# speedup vs baseline: 1.0748x; 1.0148x over previous
"""LinearShift kernel for Trainium2 (8 NeuronCores, column-parallel).

Computes: out = floor(input*2^16)*2^-16 @ (exp2(round(shift)) * sign(sign)).T
               + floor(bias*2^16)*2^-16

The correctness gate is rel_err < 2e-2 (L2).  A single bf16 matmul pass
(input RNE-rounded to bf16, weights exact powers of two in bf16) lands at
~2.4e-3 incl. bf16 output, so the device does ONE bf16 matmul per output
tile instead of the exact hi/lo pair -- half the PE work of the exact
kernel.

Strategy per core c (out_features sharded 8 x 512):
  - host: quantize+cast input to bf16 and pre-tile it so every [128,512]
    x-tile is a CONTIGUOUS 128KB block in HBM (strided 1KB-line reads
    measured only ~190GB/s; contiguous reads run much closer to the
    ~360GB/s per-core HBM limit).  Weights w = exp2(round(shift)) *
    sign(sign) computed on host (exact in bf16), sharded+tiled the same
    way; bias floor-quantized on host.
  - device: w tiles prefetched up-front on the scalar HWDGE ring while
    warmup matmuls open the HAM clock gate; x streams on the sync ring;
    out[m,n] accumulates w.T@x in PSUM over 32 k-tiles, evacuated with a
    per-partition bias add on the scalar engine to bf16, DMA'd out on
    the scalar ring.
"""
import sys
sys.path.insert(0, '/opt/trn_rl_repo')

import numpy as np
import ml_dtypes

import concourse.bass as bass
import concourse.mybir as mybir
from concourse import bacc
from concourse.tile import TileContext
from concourse.bass_utils import run_bass_kernel_spmd

F32 = mybir.dt.float32
BF16 = mybir.dt.bfloat16
FP8E5 = mybir.dt.float8e5
ACT = mybir.ActivationFunctionType
ALU = mybir.AluOpType

N_CORES = 8
TOK = 4096          # tokens (rows of input)
IN_F = 4096         # contraction dim
OUT_F = 4096        # out features
OUT_S = OUT_F // N_CORES   # 512 out features per core
KT = IN_F // 128    # 32 k-tiles
MT = OUT_S // 128   # 4 m-tiles per core
NCH = TOK // 512    # 8 token chunks of 512

_cached = {}


def _build_nc():
    nc = bacc.Bacc("TRN2", target_bir_lowering=False, num_devices=N_CORES)
    # x_t: tile (ch,k) of x.T lives at rows (ch*KT+k)*128, contiguous.
    x_t = nc.declare_dram_parameter("x_t", [NCH * KT * 128, 512], BF16,
                                    isOutput=False)
    # wT: tile k at rows k*128 (contiguous blocks already).  e5m2 -- the
    # weights are powers of two in [2^-10, 2^-1], exact in fp8 e5m2; the
    # PE allows mixed fp8-stationary x bf16-moving at full bf16 speed,
    # and this halves the weight-prefetch bytes.
    wT = nc.declare_dram_parameter("wT", [IN_F, OUT_S], FP8E5, isOutput=False)
    qb = nc.declare_dram_parameter("qb", [OUT_S], F32, isOutput=False)
    # out_t: tile (ch,m) at rows (ch*MT+m)*128, contiguous, bf16.
    out_t = nc.declare_dram_parameter("out_t", [NCH * MT * 128, 512], BF16,
                                      isOutput=True)

    with TileContext(nc) as tc, \
            tc.tile_pool(name="w", bufs=KT) as wpool, \
            tc.tile_pool(name="consts", bufs=1) as cpool, \
            tc.tile_pool(name="x", bufs=20) as xpool, \
            tc.tile_pool(name="o", bufs=4) as opool, \
            tc.tile_pool(name="p", bufs=2, space="PSUM") as ppool:

        # ---- PE warmup: dummy matmuls on scratch so the HAM clock-gate
        # opens (1.2 -> 2.4 GHz) while weights stream in.
        scratch = cpool.tile([128, 128], BF16, tag="scratch")
        nc.vector.memset(scratch, 0.0)
        warm_ps = ppool.tile([128, 128], F32, tag="ps0", name="warm_ps")
        for i in range(40):
            nc.tensor.matmul(warm_ps, scratch, scratch, start=True, stop=True)

        # ---- weights: all 32 tiles prefetched on the scalar HWDGE ring
        # (x uses the sync ring; the SDMA engines round-robin fairly).
        wt = []
        for k in range(KT):
            w_k = wpool.tile([128, OUT_S], FP8E5, tag="wt", name=f"w{k}")
            nc.scalar.dma_start(out=w_k, in_=wT[k * 128:(k + 1) * 128, :])
            wt.append(w_k)

        # ---- bias (already floor-quantized on host): qb_t[p, m] = qb[m*128+p]
        qb_t = cpool.tile([128, MT], F32, tag="qb")
        nc.sync.dma_start(
            out=qb_t, in_=qb.ap().rearrange("(m p) -> p m", p=128))

        # ---- main loop ----
        for ch in range(NCH):
            psum = [ppool.tile([128, 512], F32, tag=f"ps{m}", name=f"ps{ch}_{m}")
                    for m in range(MT)]
            for k in range(KT):
                x_tl = xpool.tile([128, 512], BF16, tag="x")
                r0 = (ch * KT + k) * 128
                nc.sync.dma_start(out=x_tl, in_=x_t[r0:r0 + 128, :])
                for m in range(MT):
                    nc.tensor.matmul(psum[m], wt[k][:, m * 128:(m + 1) * 128],
                                     x_tl, start=(k == 0), stop=(k == KT - 1))
            for m in range(MT):
                ob = opool.tile([128, 512], BF16, tag="ob")
                # split evacuation across ScalarE and VectorE so the
                # last chunk's 4 evacuations run 2-wide
                if m < 2:
                    nc.scalar.activation(ob, psum[m], ACT.Identity,
                                         bias=qb_t[:, m:m + 1], scale=1.0)
                else:
                    nc.vector.tensor_scalar(ob, psum[m], qb_t[:, m:m + 1],
                                            None, ALU.add)
                r0 = (ch * MT + m) * 128
                nc.scalar.dma_start(out=out_t[r0:r0 + 128, :], in_=ob)
    nc.finalize()
    return nc


def _make_in_maps(input, shift, sign, bias):
    """Host-side prep: quantize + cast + tile + shard. Shared by kernel()
    and the profiling path in test.py."""
    input = np.asarray(input, dtype=np.float32)
    shift = np.asarray(shift, dtype=np.float32)
    sign = np.asarray(sign, dtype=np.float32)
    bias = np.asarray(bias, dtype=np.float32)

    # q_in = clip(floor(x*2^16)*2^-16, -2^15, 2^15-1), RNE-cast to bf16
    t = np.floor(input * 65536.0)
    np.clip(t, -2147483648.0, 2147418112.0, out=t)
    xb = (t * np.float32(2.0 ** -16)).astype(ml_dtypes.bfloat16)
    # pre-tile: x_t[(ch*KT+k)*128 + p, t] = x.T[k*128+p, ch*512+t]
    #         = xb[ch*512+t, k*128+p]
    x_t = np.ascontiguousarray(
        xb.reshape(NCH, 512, KT, 128).transpose(0, 2, 3, 1)
    ).reshape(NCH * KT * 128, 512)

    # w = exp2(round(shift)) * sign(clip(sign,-1,1)) -- exact in fp8 e5m2
    v = np.exp2(np.round(shift)) * np.sign(np.clip(sign, -1.0, 1.0))
    vT = np.ascontiguousarray(v.T.astype(ml_dtypes.float8_e5m2))

    qbias = np.clip(np.floor(bias * 65536.0) * np.float32(2.0 ** -16),
                    -32768.0, 32767.0).astype(np.float32)

    in_maps = []
    for c in range(N_CORES):
        sl = slice(c * OUT_S, (c + 1) * OUT_S)
        in_maps.append({
            "x_t": x_t,
            "wT": np.ascontiguousarray(vT[:, sl]),
            "qb": np.ascontiguousarray(qbias[sl]),
        })
    return in_maps


def kernel(input, shift, sign, bias):
    if "nc" not in _cached:
        _cached["nc"] = _build_nc()
    nc = _cached["nc"]

    in_maps = _make_in_maps(input, shift, sign, bias)
    res = run_bass_kernel_spmd(nc, in_maps, list(range(N_CORES))).results
    # out_t[(ch*MT+m)*128 + p, t] -> out[ch*512+t, c*512 + m*128+p]
    cols = []
    for c in range(N_CORES):
        a = res[c]["out_t"].astype(np.float32)
        cols.append(a.reshape(NCH, MT, 128, 512).transpose(0, 3, 1, 2)
                    .reshape(TOK, OUT_S))
    return np.ascontiguousarray(np.concatenate(cols, axis=1))


if __name__ == "__main__":
    rng = np.random.default_rng(0)
    inputs = {
        "input": rng.standard_normal((TOK, IN_F)).astype(np.float32),
        "shift": rng.uniform(-10, -1, (OUT_F, IN_F)).astype(np.float32),
        "sign": rng.uniform(-1, 0, (OUT_F, IN_F)).astype(np.float32),
        "bias": rng.uniform(-1 / 64, 1 / 64, OUT_F).astype(np.float32),
    }
    out = kernel(**inputs)
    print("out", out.shape, out.dtype, out[:2, :4])
